# revision 3
# baseline (speedup 1.0000x reference)
"""Trainium2 Bass kernel: 1024-point FFT of real rows -> (real, imag).

Math: out = FFT_1024(x[b, :]) for each row. Exploits real-input symmetry:
  U[n] = x[n] + x[1024-n], V[n] = x[n] - x[1024-n]  (n in [1,512))
  Xr[k] = sum_{n<512} U[n] cos(2pi n k/1024) + (-1)^k x[512]
  Xi[k] = -sum_{n<512} V[n] sin(2pi n k/1024)
  X[1024-k] = conj(X[k])  -> compute k in [0,512], mirror the rest.

Per core (2048 rows): DVE builds U/V, PE transposes 128x128 tiles, then
K=512 fp32r matmuls against hardcoded cos/-sin matrices, and the staging
pass applies the rank-1 (+/- x[512]) correction and writes the mirrored
conjugate half. Pure data-parallel across 8 cores, no collectives.
"""

import os
import numpy as np

N_FFT = 1024
BATCH = 16384
N_CORES = 8
B_CORE = BATCH // N_CORES  # 2048
P = 128
HALF = 512

_BUILD_CACHE = {}


def _constants():
    n = np.arange(HALF, dtype=np.float64)
    k = np.arange(1, HALF + 1, dtype=np.float64)
    ang = (2.0 * np.pi / N_FFT) * np.outer(n, k)
    cr = np.cos(ang).astype(np.float32)          # [512, 512], col c -> freq k=c+1
    ci = (-np.sin(ang)).astype(np.float32)
    ci[0, :] = 0.0                               # V[:,0] is garbage; kill its row
    return np.ascontiguousarray(cr), np.ascontiguousarray(ci)


def build_nc(b_core=B_CORE):
    """Build + compile the per-core Bass program (same NEFF on all cores)."""
    import concourse.mybir as mybir
    import concourse.tile as tile
    from concourse import bacc
    from concourse.masks import make_identity

    f32 = mybir.dt.float32
    f32r = mybir.dt.float32r

    nc = bacc.Bacc(
        "TRN2", target_bir_lowering=False, debug=False, num_devices=N_CORES
    )

    x_in = nc.dram_tensor("x", [b_core, N_FFT], f32, kind="ExternalInput")
    cr_in = nc.dram_tensor("cr", [HALF, HALF], f32r, kind="ExternalInput")
    ci_in = nc.dram_tensor("ci", [HALF, HALF], f32r, kind="ExternalInput")
    o_r = nc.dram_tensor("out_r", [b_core, N_FFT], f32, kind="ExternalOutput")
    o_i = nc.dram_tensor("out_i", [b_core, N_FFT], f32, kind="ExternalOutput")

    n_tiles = b_core // P

    with tile.TileContext(nc) as tc:
        with (
            tc.tile_pool(name="const", bufs=1) as cpool,
            tc.tile_pool(name="work", bufs=3) as wpool,
            tc.tile_pool(name="outp", bufs=3) as opool,
            tc.tile_pool(name="pst", bufs=2, space="PSUM") as pst,
            tc.tile_pool(name="psm", bufs=2, space="PSUM") as psm,
        ):
            ident = cpool.tile([P, P], f32)
            make_identity(nc, ident)
            cr_sb = cpool.tile([P, 4, HALF], f32r)
            ci_sb = cpool.tile([P, 4, HALF], f32r)
            nc.sync.dma_start(
                out=cr_sb, in_=cr_in.ap().rearrange("(c p) k -> p c k", p=P)
            )
            nc.sync.dma_start(
                out=ci_sb, in_=ci_in.ap().rearrange("(c p) k -> p c k", p=P)
            )

            for t in range(n_tiles):
                rows = slice(t * P, (t + 1) * P)
                xt = wpool.tile([P, N_FFT], f32, tag="xt")
                nc.sync.dma_start(out=xt[:], in_=x_in[rows, :])

                uv = wpool.tile([P, 2, HALF], f32, tag="uv")
                u = uv[:, 0]
                v = uv[:, 1]
                rev = xt[:, 1023:HALF:-1]  # cols 1023..513
                nc.vector.tensor_add(out=u[:, 1:HALF], in0=xt[:, 1:HALF], in1=rev)
                nc.vector.tensor_sub(out=v[:, 1:HALF], in0=xt[:, 1:HALF], in1=rev)
                nc.scalar.copy(out=u[:, 0:1], in_=xt[:, 0:1])
                nc.scalar.copy(out=v[:, 0:1], in_=xt[:, 0:1])
                x512 = xt[:, HALF : HALF + 1]

                # row-sum of U for the k=0 column
                rsum = wpool.tile([P, 1], f32, tag="rsum")
                nc.vector.reduce_sum(out=rsum[:], in_=u[:, 0:HALF], axis=mybir.AxisListType.X)

                # PE transposes: U,V [128b x 512n] -> [128n x 128b] chunks
                utp = pst.tile([P, HALF], f32, tag="utp")
                vtp = pst.tile([P, HALF], f32, tag="vtp")
                for j in range(4):
                    cs = slice(j * P, (j + 1) * P)
                    nc.tensor.transpose(utp[:, cs], u[:, cs], ident)
                    nc.tensor.transpose(vtp[:, cs], v[:, cs], ident)
                ut = wpool.tile([P, 4, P], f32r, tag="ut")
                vt = wpool.tile([P, 4, P], f32r, tag="vt")
                for j in range(4):
                    cs = slice(j * P, (j + 1) * P)
                    nc.scalar.copy(out=ut[:, j], in_=utp[:, cs])
                    nc.vector.tensor_copy(out=vt[:, j], in_=vtp[:, cs])

                # K=512 matmuls (fp32r single-pass): psum col c -> freq k=c+1
                pr = psm.tile([P, HALF], f32, tag="pr")
                pi = psm.tile([P, HALF], f32, tag="pi")
                for j in range(4):
                    nc.tensor.matmul(
                        pr[:],
                        lhsT=ut[:, j],
                        rhs=cr_sb[:, j],
                        start=(j == 0),
                        stop=(j == 3),
                    )
                for j in range(4):
                    nc.tensor.matmul(
                        pi[:],
                        lhsT=vt[:, j],
                        rhs=ci_sb[:, j],
                        start=(j == 0),
                        stop=(j == 3),
                    )

                # staging with rank-1 (+/- x512) and conjugate mirror
                orr = opool.tile([P, N_FFT], f32, tag="orr")
                oii = opool.tile([P, N_FFT], f32, tag="oii")
                # real, k in [1,513): odd k subtract x512, even k add
                nc.vector.tensor_scalar_sub(orr[:, 1:512:2], pr[:, 0:512:2], x512)
                nc.vector.tensor_scalar_add(orr[:, 2:514:2], pr[:, 1:512:2], x512)
                # real mirror, k in [513,1024)
                nc.vector.tensor_scalar_sub(orr[:, 513:1024:2], pr[:, 510::-2], x512)
                nc.vector.tensor_scalar_add(orr[:, 514:1024:2], pr[:, 509:0:-2], x512)
                # real k=0 = sum(U) + x512
                nc.vector.tensor_scalar_add(orr[:, 0:1], rsum[:], x512)
                # imag
                nc.vector.memset(oii[:, 0:1], 0.0)
                nc.scalar.copy(out=oii[:, 1:513], in_=pi[:, 0:512])
                nc.scalar.mul(oii[:, 513:1024], pi[:, 510::-1], -1.0)

                nc.sync.dma_start(out=o_r[rows, :], in_=orr[:])
                nc.sync.dma_start(out=o_i[rows, :], in_=oii[:])

    nc.compile()
    return nc


def _get_nc(b_core=B_CORE):
    if b_core not in _BUILD_CACHE:
        _BUILD_CACHE[b_core] = build_nc(b_core)
    return _BUILD_CACHE[b_core]


def kernel(**inputs):
    from concourse.bass_utils import run_bass_kernel_spmd

    x = np.ascontiguousarray(np.asarray(inputs["x"], dtype=np.float32))
    assert x.shape == (BATCH, N_FFT), x.shape
    cr, ci = _constants()
    nc = _get_nc()
    shards = np.split(x, N_CORES, axis=0)
    in_maps = [{"x": s, "cr": cr, "ci": ci} for s in shards]
    trace = bool(int(os.environ.get("FFT_KERNEL_TRACE", "0")))
    res = run_bass_kernel_spmd(
        nc, in_maps, core_ids=list(range(N_CORES)), trace=trace
    )
    if trace:
        kernel.last_results = res
    real = np.concatenate([res.results[c]["out_r"] for c in range(N_CORES)], axis=0)
    imag = np.concatenate([res.results[c]["out_i"] for c in range(N_CORES)], axis=0)
    return real, imag


# revision 7
# speedup vs baseline: 1.2517x; 1.2517x over previous
"""Trainium2 Bass kernel: 1024-point FFT of real rows -> (real, imag).

Math: out = FFT_1024(x[b, :]) per row, via real-input symmetry:
  U[n] = x[n] + x[1024-n], V[n] = x[n] - x[1024-n]   (n in [1,512))
  Xr[k] = sum_{n<512} U[n] cos(2pi n k/1024) + (-1)^k x[512]
  Xi[k] = -sum_{n<512} V[n] sin(2pi n k/1024)
  X[1024-k] = conj(X[k])  -> compute k in [1,513), mirror k in [513,1024),
  k=0 column done host-side (row sum).

The host ships U^T / V^T (same byte count as x) so the device needs no
transposes: per 128-row tile, 4+1 / 4 accumulating K=512 fp32r matmuls
produce Xr/Xi for k in [1,513) in PSUM (the 5th matmul adds the rank-1
(-1)^k x[512] term via a one-hot lhsT). Those PSUM halves DMA straight
to DRAM; DVE/ACT only write the reversed conjugate-mirror halves.
Pure data-parallel across 8 cores, no collectives.
"""

import os
import numpy as np

N_FFT = 1024
BATCH = 16384
N_CORES = 8
B_CORE = BATCH // N_CORES  # 2048
P = 128
HALF = 512

_BUILD_CACHE = {}


def _constants():
    n = np.arange(HALF, dtype=np.float64)
    k = np.arange(1, HALF + 1, dtype=np.float64)
    ang = (2.0 * np.pi / N_FFT) * np.outer(n, k)
    cr = np.cos(ang).astype(np.float32)          # [512, 512], col c -> freq k=c+1
    ci = (-np.sin(ang)).astype(np.float32)
    ci[0, :] = 0.0                               # V^T row 0 is garbage; kill it
    # alternating (-1)^k row for the rank-1 x[512] term (k=c+1: -1 at even c)
    alt = np.zeros((P, HALF), dtype=np.float32)
    alt[0, 0::2] = -1.0
    alt[0, 1::2] = 1.0
    return np.ascontiguousarray(cr), np.ascontiguousarray(ci), alt


def build_nc(b_core=B_CORE):
    """Build + compile the per-core Bass program (same NEFF on all cores)."""
    import concourse.mybir as mybir
    import concourse.tile as tile
    from concourse import bacc

    f32 = mybir.dt.float32
    f32r = mybir.dt.float32r

    nc = bacc.Bacc(
        "TRN2", target_bir_lowering=False, debug=False, num_devices=N_CORES
    )

    ut_in = nc.dram_tensor("ut", [HALF, b_core], f32r, kind="ExternalInput")
    vt_in = nc.dram_tensor("vt", [HALF, b_core], f32r, kind="ExternalInput")
    cr_in = nc.dram_tensor("cr", [HALF, HALF], f32r, kind="ExternalInput")
    ci_in = nc.dram_tensor("ci", [HALF, HALF], f32r, kind="ExternalInput")
    alt_in = nc.dram_tensor("alt", [P, HALF], f32r, kind="ExternalInput")
    o_r = nc.dram_tensor("out_r", [b_core, N_FFT], f32, kind="ExternalOutput")
    o_i = nc.dram_tensor("out_i", [b_core, N_FFT], f32, kind="ExternalOutput")

    GC = min(HALF, b_core)          # b-columns per DMA group
    n_groups = b_core // GC
    n_sub = GC // P                 # 128-row tiles per group

    ut_r = ut_in.ap().rearrange("(j p) b -> p j b", p=P)
    vt_r = vt_in.ap().rearrange("(j p) b -> p j b", p=P)

    with tile.TileContext(nc) as tc:
        with (
            tc.tile_pool(name="const", bufs=1) as cpool,
            tc.tile_pool(name="work", bufs=2) as wpool,
            tc.tile_pool(name="outp", bufs=4) as opool,
            tc.tile_pool(name="psm", bufs=3, space="PSUM") as psm,
        ):
            cr_sb = cpool.tile([P, 4, HALF], f32r)
            ci_sb = cpool.tile([P, 4, HALF], f32r)
            alt_sb = cpool.tile([P, HALF], f32r)
            nc.sync.dma_start(
                out=cr_sb, in_=cr_in.ap().rearrange("(c p) k -> p c k", p=P)
            )
            nc.sync.dma_start(
                out=ci_sb, in_=ci_in.ap().rearrange("(c p) k -> p c k", p=P)
            )
            nc.sync.dma_start(out=alt_sb, in_=alt_in.ap())

            for g in range(n_groups):
                gcols = slice(g * GC, (g + 1) * GC)
                ut_sb = wpool.tile([P, 4, GC], f32r, tag="ut")
                vt_sb = wpool.tile([P, 4, GC], f32r, tag="vt")
                nc.sync.dma_start(out=ut_sb, in_=ut_r[:, :, gcols])
                nc.sync.dma_start(out=vt_sb, in_=vt_r[:, :, gcols])

                for s in range(n_sub):
                    t = g * n_sub + s
                    rows = slice(t * P, (t + 1) * P)
                    bsl = slice(s * P, (s + 1) * P)

                    pr = psm.tile([P, HALF], f32, tag="pr")
                    pi = psm.tile([P, HALF], f32, tag="pi")
                    for j in range(4):
                        nc.tensor.matmul(
                            pr[:], lhsT=ut_sb[:, j, bsl], rhs=cr_sb[:, j],
                            start=(j == 0), stop=False,
                        )
                    # rank-1 (-1)^k x[512]: V^T row 0 holds x[512] (Ci row 0
                    # is zero, so it cannot pollute pi); alt is one-hot row 0
                    nc.tensor.matmul(
                        pr[:], lhsT=vt_sb[:, 0, bsl], rhs=alt_sb[:],
                        start=False, stop=True,
                    )
                    for j in range(4):
                        nc.tensor.matmul(
                            pi[:], lhsT=vt_sb[:, j, bsl], rhs=ci_sb[:, j],
                            start=(j == 0), stop=(j == 3),
                        )

                    # stage cols [1,1024): [1,513) straight, [513,1024) mirrored
                    orr = opool.tile([P, N_FFT - 1], f32, tag="orr")
                    oii = opool.tile([P, N_FFT - 1], f32, tag="oii")
                    nc.vector.tensor_copy(out=orr[:, 0:512], in_=pr[:])
                    nc.vector.tensor_copy(out=orr[:, 512:1023], in_=pr[:, 510::-1])
                    nc.scalar.copy(out=oii[:, 0:512], in_=pi[:])
                    nc.scalar.mul(oii[:, 512:1023], pi[:, 510::-1], -1.0)
                    nc.sync.dma_start(out=o_r[rows, 1:1024], in_=orr[:])
                    nc.sync.dma_start(out=o_i[rows, 1:1024], in_=oii[:])

    nc.compile()
    return nc


def _get_nc(b_core=B_CORE):
    if b_core not in _BUILD_CACHE:
        _BUILD_CACHE[b_core] = build_nc(b_core)
    return _BUILD_CACHE[b_core]


def _host_prep(x):
    """U/V (real-FFT fold) in transposed layout + host-side k=0 column."""
    B = x.shape[0]
    U = np.empty((B, HALF), dtype=np.float32)
    V = np.empty((B, HALF), dtype=np.float32)
    U[:, 0] = x[:, 0]
    V[:, 0] = x[:, HALF]          # dead slot rides along for the rank-1 term
    rev = x[:, 1023:HALF:-1]
    np.add(x[:, 1:HALF], rev, out=U[:, 1:HALF])
    np.subtract(x[:, 1:HALF], rev, out=V[:, 1:HALF])
    col0 = (U.sum(axis=1, dtype=np.float64) + x[:, HALF]).astype(np.float32)
    ut = np.ascontiguousarray(U.T)               # [512, B]
    vt = np.ascontiguousarray(V.T)
    return ut, vt, col0


def kernel(**inputs):
    from concourse.bass_utils import run_bass_kernel_spmd

    x = np.ascontiguousarray(np.asarray(inputs["x"], dtype=np.float32))
    assert x.shape == (BATCH, N_FFT), x.shape
    cr, ci, alt = _constants()
    ut, vt, col0 = _host_prep(x)
    nc = _get_nc()
    in_maps = []
    for c in range(N_CORES):
        sl = slice(c * B_CORE, (c + 1) * B_CORE)
        in_maps.append(
            {
                "ut": np.ascontiguousarray(ut[:, sl]),
                "vt": np.ascontiguousarray(vt[:, sl]),
                "cr": cr,
                "ci": ci,
                "alt": alt,
            }
        )
    trace = bool(int(os.environ.get("FFT_KERNEL_TRACE", "0")))
    res = run_bass_kernel_spmd(
        nc, in_maps, core_ids=list(range(N_CORES)), trace=trace
    )
    if trace:
        kernel.last_results = res
    real = np.concatenate([res.results[c]["out_r"] for c in range(N_CORES)], axis=0)
    imag = np.concatenate([res.results[c]["out_i"] for c in range(N_CORES)], axis=0)
    real[:, 0] = col0
    imag[:, 0] = 0.0
    return real, imag


# revision 8
# speedup vs baseline: 1.3848x; 1.1064x over previous
"""Trainium2 Bass kernel: 1024-point FFT of real rows -> (real, imag).

Math: out = FFT_1024(x[b, :]) per row, via real-input symmetry:
  U[n] = x[n] + x[1024-n], V[n] = x[n] - x[1024-n]   (n in [1,512))
  Xr[k] = sum_{n<512} U[n] cos(2pi n k/1024) + (-1)^k x[512]
  Xi[k] = -sum_{n<512} V[n] sin(2pi n k/1024)
  X[1024-k] = conj(X[k])  -> compute k in [1,513), mirror k in [513,1024),
  k=0 column done host-side (row sum).

The host ships U^T / V^T (same byte count as x, group-blocked) so the
device needs no transposes: per 128-row tile, 4+1 / 4 accumulating K=512
fp32r matmuls produce Xr/Xi for k in [1,513) in PSUM; the 5th matmul
adds the rank-1 (-1)^k x[512] term by reading V^T's dead row 0 (Ci row 0
is zero) against a one-hot `alt` matrix. DVE/ACT stage the straight +
conjugate-mirrored halves into group tiles.

All DMAs are shaped for one long contiguous run per partition: batch
rows are interleaved across PSUM partitions (row b = gstart + 4*m + s)
so each output partition writes 4 adjacent 4KB DRAM rows (16KB runs),
and inputs are host-blocked per group (32KB runs). Input DMAs ride the
sync queue, output DMAs the gpsimd queue. Pure data-parallel across 8
cores, no collectives.
"""

import os
import numpy as np

N_FFT = 1024
BATCH = 16384
N_CORES = 8
B_CORE = BATCH // N_CORES  # 2048
P = 128
HALF = 512
GC = 512                   # batch rows per group

_BUILD_CACHE = {}


def _constants():
    n = np.arange(HALF, dtype=np.float64)
    k = np.arange(1, HALF + 1, dtype=np.float64)
    ang = (2.0 * np.pi / N_FFT) * np.outer(n, k)
    cr = np.cos(ang).astype(np.float32)          # [512, 512], col c -> freq k=c+1
    ci = (-np.sin(ang)).astype(np.float32)
    ci[0, :] = 0.0                               # V^T row 0 carries x[512]; kill it
    # alternating (-1)^k row for the rank-1 x[512] term (k=c+1: -1 at even c)
    alt = np.zeros((P, HALF), dtype=np.float32)
    alt[0, 0::2] = -1.0
    alt[0, 1::2] = 1.0
    return np.ascontiguousarray(cr), np.ascontiguousarray(ci), alt


def build_nc(b_core=B_CORE):
    """Build + compile the per-core Bass program (same NEFF on all cores)."""
    import concourse.mybir as mybir
    import concourse.tile as tile
    from concourse import bacc

    f32 = mybir.dt.float32
    f32r = mybir.dt.float32r

    gc = min(GC, b_core)
    n_groups = b_core // gc
    n_sub = gc // P            # 128-row tiles per group

    nc = bacc.Bacc(
        "TRN2", target_bir_lowering=False, debug=False, num_devices=N_CORES
    )

    ut_in = nc.dram_tensor("ut", [n_groups, HALF, gc], f32r, kind="ExternalInput")
    vt_in = nc.dram_tensor("vt", [n_groups, HALF, gc], f32r, kind="ExternalInput")
    cr_in = nc.dram_tensor("cr", [HALF, HALF], f32r, kind="ExternalInput")
    ci_in = nc.dram_tensor("ci", [HALF, HALF], f32r, kind="ExternalInput")
    alt_in = nc.dram_tensor("alt", [P, HALF], f32r, kind="ExternalInput")
    o_r = nc.dram_tensor("out_r", [b_core, N_FFT], f32, kind="ExternalOutput")
    o_i = nc.dram_tensor("out_i", [b_core, N_FFT], f32, kind="ExternalOutput")

    # chunk j / partition p hold row n = 4p+j of U^T,V^T (and matching C row)
    ut_r = ut_in.ap().rearrange("g (p j) b -> g p j b", j=4)
    vt_r = vt_in.ap().rearrange("g (p j) b -> g p j b", j=4)

    with tile.TileContext(nc) as tc:
        with (
            tc.tile_pool(name="const", bufs=1) as cpool,
            tc.tile_pool(name="work", bufs=2) as wpool,
            tc.tile_pool(name="outp", bufs=2) as opool,
            tc.tile_pool(name="psm", bufs=3, space="PSUM") as psm,
        ):
            cr_sb = cpool.tile([P, 4, HALF], f32r)
            ci_sb = cpool.tile([P, 4, HALF], f32r)
            alt_sb = cpool.tile([P, HALF], f32r)
            nc.sync.dma_start(
                out=cr_sb, in_=cr_in.ap().rearrange("(p j) k -> p j k", j=4)
            )
            nc.sync.dma_start(
                out=ci_sb, in_=ci_in.ap().rearrange("(p j) k -> p j k", j=4)
            )
            nc.sync.dma_start(out=alt_sb, in_=alt_in.ap())

            for g in range(n_groups):
                ut_sb = wpool.tile([P, 4, gc], f32r, tag="ut")
                vt_sb = wpool.tile([P, 4, gc], f32r, tag="vt")
                nc.sync.dma_start(out=ut_sb, in_=ut_r[g])
                nc.sync.dma_start(out=vt_sb, in_=vt_r[g])

                org = opool.tile([P, n_sub, N_FFT], f32, tag="org")
                oig = opool.tile([P, n_sub, N_FFT], f32, tag="oig")
                nc.vector.memset(org[:, :, 0:1], 0.0)   # junk col 0 (host fixes)
                nc.vector.memset(oig[:, :, 0:1], 0.0)

                for s in range(n_sub):
                    # psum partition m <-> batch row gstart + n_sub*m + s
                    bsl = slice(s, gc, n_sub)
                    pr = psm.tile([P, HALF], f32, tag="pr")
                    pi = psm.tile([P, HALF], f32, tag="pi")
                    for j in range(4):
                        nc.tensor.matmul(
                            pr[:], lhsT=ut_sb[:, j, bsl], rhs=cr_sb[:, j],
                            start=(j == 0), stop=False,
                        )
                    # rank-1 (-1)^k x[512] via V^T row 0 against one-hot alt
                    nc.tensor.matmul(
                        pr[:], lhsT=vt_sb[:, 0, bsl], rhs=alt_sb[:],
                        start=False, stop=True,
                    )
                    for j in range(4):
                        nc.tensor.matmul(
                            pi[:], lhsT=vt_sb[:, j, bsl], rhs=ci_sb[:, j],
                            start=(j == 0), stop=(j == 3),
                        )

                    # stage cols [1,513) straight, [513,1024) mirrored
                    nc.vector.tensor_copy(out=org[:, s, 1:513], in_=pr[:])
                    nc.vector.tensor_copy(out=org[:, s, 513:1024], in_=pr[:, 510::-1])
                    nc.scalar.copy(out=oig[:, s, 1:513], in_=pi[:])
                    nc.scalar.mul(oig[:, s, 513:1024], pi[:, 510::-1], -1.0)

                rows = slice(g * gc, (g + 1) * gc)
                nc.gpsimd.dma_start(
                    out=o_r[rows, :].rearrange("(p s) k -> p s k", s=n_sub),
                    in_=org[:],
                )
                nc.gpsimd.dma_start(
                    out=o_i[rows, :].rearrange("(p s) k -> p s k", s=n_sub),
                    in_=oig[:],
                )

    nc.compile()
    return nc


def _get_nc(b_core=B_CORE):
    if b_core not in _BUILD_CACHE:
        _BUILD_CACHE[b_core] = build_nc(b_core)
    return _BUILD_CACHE[b_core]


def _host_prep(x):
    """U/V (real-FFT fold) in transposed layout + host-side k=0 column."""
    B = x.shape[0]
    U = np.empty((B, HALF), dtype=np.float32)
    V = np.empty((B, HALF), dtype=np.float32)
    U[:, 0] = x[:, 0]
    V[:, 0] = x[:, HALF]          # dead slot rides along for the rank-1 term
    rev = x[:, 1023:HALF:-1]
    np.add(x[:, 1:HALF], rev, out=U[:, 1:HALF])
    np.subtract(x[:, 1:HALF], rev, out=V[:, 1:HALF])
    col0 = (U.sum(axis=1, dtype=np.float64) + x[:, HALF]).astype(np.float32)
    ut = np.ascontiguousarray(U.T)               # [512, B]
    vt = np.ascontiguousarray(V.T)
    return ut, vt, col0


def _blocked(a_t, sl, b_core):
    """[512, B] column-slice -> group-blocked [n_groups, 512, gc] contiguous."""
    gc = min(GC, b_core)
    n_groups = b_core // gc
    s = a_t[:, sl]
    return np.ascontiguousarray(s.reshape(HALF, n_groups, gc).transpose(1, 0, 2))


def kernel(**inputs):
    from concourse.bass_utils import run_bass_kernel_spmd

    x = np.ascontiguousarray(np.asarray(inputs["x"], dtype=np.float32))
    assert x.shape == (BATCH, N_FFT), x.shape
    cr, ci, alt = _constants()
    ut, vt, col0 = _host_prep(x)
    nc = _get_nc()
    in_maps = []
    for c in range(N_CORES):
        sl = slice(c * B_CORE, (c + 1) * B_CORE)
        in_maps.append(
            {
                "ut": _blocked(ut, sl, B_CORE),
                "vt": _blocked(vt, sl, B_CORE),
                "cr": cr,
                "ci": ci,
                "alt": alt,
            }
        )
    trace = bool(int(os.environ.get("FFT_KERNEL_TRACE", "0")))
    res = run_bass_kernel_spmd(
        nc, in_maps, core_ids=list(range(N_CORES)), trace=trace
    )
    if trace:
        kernel.last_results = res
    real = np.concatenate([res.results[c]["out_r"] for c in range(N_CORES)], axis=0)
    imag = np.concatenate([res.results[c]["out_i"] for c in range(N_CORES)], axis=0)
    real[:, 0] = col0
    imag[:, 0] = 0.0
    return real, imag
